# revision 7
# baseline (speedup 1.0000x reference)
"""v3 Trainium2 Bass kernel.

Key changes vs v2 (327us):
 - The per-batch AllGather collective (15us constant in the cost model, ~20us
   with DMA hops) is replaced by direct core-to-core remote_dma sends of the
   pre-activation vector, with an XOR-rotation layout: core c sends its
   replicated aff column to peer P(c)^d, landing in column d of the
   receiver's rcv tile. The receiver assembles the span vector with one
   mult+reduce against a 0/1 selection mask. The rank-dependent permutation
   this induces is folded into the per-core weight/mask/param layouts on the
   host (physical slot p holds logical neuron 16*Pinv[P(r)^(p>>4)] + p%16).
 - All big weights stream as bf16 (half the DMA bytes, 1 PE cycle/row vs 4
   for fp32) on the SP queue, which has no critical work.
 - Neuron scores are built as 16 rank-1 outer products k_t (x) q_t from
   transposed q/k rows, removing the per-neuron k-scale pass on DVE (the
   1/sqrt(S) scale is folded into Wq on the host).
"""
import sys
import numpy as np

sys.path.insert(0, "/opt/trn_rl_repo")

I, L, T, S = 128, 8, 128, 128
N_CORES = 8
TL = T // N_CORES
EPS = 1e-5
RS = float(1.0 / np.sqrt(np.float32(S)))
GC = 0.7978845608028654
GA = 0.044715
MAGIC = 0x5F3759DF

_cached = None
_P = None


def _nc_perm():
    """Logical core -> physical nc index, with fake-nrt fallback."""
    global _P
    if _P is not None:
        return _P
    from concourse import libnrt, bass_interp
    try:
        m = libnrt.get_trn2_nc_mapping()
        _P = [m[(0, i)] for i in range(8)]
        return _P
    except Exception:
        pass
    base = libnrt._TRN2_NC_BASE
    mapping = {(d, i): base[i] for d in range(64) for i in range(8)}
    libnrt.get_trn2_nc_mapping = lambda: mapping
    libnrt.nc_to_real_nc.cache_clear()
    rid_map = {d: d for d in range(16)}
    libnrt.get_device_id_to_routing_id_mapping = lambda: rid_map
    bass_interp.get_device_id_to_routing_id_mapping = lambda: rid_map
    _P = list(base)
    return _P


def _build():
    from concourse import bacc, tile, mybir

    fp32 = mybir.dt.float32
    bf16 = mybir.dt.bfloat16
    int32 = mybir.dt.int32
    Exp = mybir.ActivationFunctionType.Exp
    Tanh = mybir.ActivationFunctionType.Tanh
    mul_op = mybir.AluOpType.mult
    add_op = mybir.AluOpType.add
    sub_op = mybir.AluOpType.subtract
    shr_op = mybir.AluOpType.arith_shift_right
    bypass = mybir.AluOpType.bypass
    div_op = mybir.AluOpType.divide
    and_op = mybir.AluOpType.bitwise_and
    or_op = mybir.AluOpType.bitwise_or
    AX = mybir.AxisListType.X

    nc = bacc.Bacc("TRN2", target_bir_lowering=False, debug=False,
                   enable_asserts=True, num_devices=N_CORES,
                   monotonic_sem_count=51)

    # tqkv: [b][s_in][(m,t)][s_out] bf16; m=0 block doubles as Q-broadcast
    # moving operand, m=1,2 blocks are k/v matvec stationaries (q pre-scaled rs)
    tqkv_d = nc.dram_tensor("tqkv", [L, S, 3 * TL * S], bf16,
                            kind="ExternalInput").ap()
    bqr_d = nc.dram_tensor("bqr", [L, 1, TL * S], bf16,
                           kind="ExternalInput").ap()
    # aux: 0:384 topo_wt | then former small: 384:416 kvb | 432:448 mt
    #   | 448:464 wmt | 464:467 topo_c | 467:470 topo_bp | 470 gamma
    #   | 471 beta | 473 g1h | 474 wbc | 476:484 selg
    aux_d = nc.dram_tensor("aux", [L, S, 484], bf16,
                           kind="ExternalInput").ap()
    pre_d = nc.dram_tensor("pre", [S, 522], fp32, kind="ExternalInput").ap()
    identb_d = nc.dram_tensor("identb", [S, S], bf16, kind="ExternalInput").ap()
    magic_d = nc.dram_tensor("magic", [1, 2], int32, kind="ExternalInput").ap()
    out_d = nc.dram_tensor("out", [TL, 1], fp32, kind="ExternalOutput").ap()

    # one remote sem per (batch, sender-delta): each is bumped exactly once
    # (by +4), so no update ever passes beyond a wait threshold
    rsems = {(b, d): nc.monotonic_semaphore(7 * b + d - 1).sem()
             for b in range(7) for d in range(1, 8)}
    lsem_q = [nc.monotonic_semaphore(49).sem(), nc.monotonic_semaphore(50).sem()]

    from concourse.bass import RemoteDMATransfer
    from concourse.instruction_name_ordered_set import InstructionNameOrderedSet

    asm_waits = []  # (BassInstruction, wait value) patched post-schedule

    with tile.TileContext(nc) as tc:
        with tc.tile_pool(name="wpool", bufs=3) as wpool, \
             tc.tile_pool(name="spool", bufs=3) as spool, \
             tc.tile_pool(name="fixed", bufs=1) as fixed, \
             tc.tile_pool(name="work", bufs=1) as work, \
             tc.tile_pool(name="ps_big", bufs=1, space="PSUM") as ps_big, \
             tc.tile_pool(name="ps_sm", bufs=1, space="PSUM") as ps_sm:

            identb = fixed.tile([S, S], bf16)
            nc.scalar.dma_start(identb[:], identb_d)
            pre = fixed.tile([S, 522], fp32)
            nc.sync.dma_start(pre[:], pre_d)
            magic = fixed.tile([1, 2], int32)
            nc.scalar.dma_start(magic[:], magic_d)
            ones_col = fixed.tile([S, 1], fp32)
            nc.vector.memset(ones_col[:], 1.0)
            ones_row = fixed.tile([1, S], fp32)
            nc.vector.memset(ones_row[:], 1.0)
            ones_row_bf = fixed.tile([1, S], bf16)
            nc.vector.memset(ones_row_bf[:], 1.0)

            cnt_regs = [nc.gpsimd.alloc_register(f"trigcnt{b}")
                        for b in range(L - 1)]

            # persistent work tiles
            v_col = work.tile([S, 1], fp32)
            v_bf = work.tile([S, 1], bf16)
            u_col = work.tile([S, 1], fp32)
            up_col = work.tile([S, 1], fp32)
            up_bf = work.tile([S, 1], bf16)
            sc = work.tile([1, 8], fp32)
            sci = sc[:].bitcast(int32)
            yA = work.tile([1, 1], fp32)
            yB = work.tile([1, 1], fp32)
            yAi = yA[:].bitcast(int32)
            yBi = yB[:].bitcast(int32)
            bc_sb = work.tile([S, 3], fp32)
            qkvt_bf = work.tile([S, 3], bf16)
            qkrow = work.tile([1, 2 * S], bf16)
            pvr_t = work.tile([S, 2], bf16)
            nc.vector.memset(pvr_t[:, 1:2], 1.0)
            Et_sb = work.tile([S, S], bf16)
            Ub = work.tile([S, S], bf16)
            qkv_sb = work.tile([S, 32], bf16)
            pvr_n = work.tile([S, 2 * TL], bf16)
            sc_ch = [work.tile([S, 512], bf16, name=f"scch{i}")
                     for i in range(4)]
            E_ch = [work.tile([S, 512], bf16, name=f"ech{i}")
                    for i in range(4)]
            rden = work.tile([S, TL], fp32)
            zp = work.tile([S, TL], fp32)
            zsum = work.tile([S, TL], fp32)
            af_big = work.tile([S, 128], fp32)

            aff_sb = [work.tile([S, 1], fp32, name=f"aff{b}") for b in range(L)]
            cnt_cells = [work.tile([1, 1], int32, name=f"cnt{b}")
                         for b in range(L - 1)]
            rcv = [work.tile([S, 8], fp32, name=f"rcv{b}") for b in range(L - 1)]

            # PSUM: four independent score banks; small stuff in one bank
            sbank = [ps_big.tile([S, 512], fp32, name=f"sbank{i}")
                     for i in range(4)]
            smps = ps_sm.tile([S, 512], fp32)
            qkv_ps = smps[:, 0:48]
            pvn_ps = smps[:, 48:80]
            A_ps = smps[:, 80:83]
            bc_ps = smps[:, 84:87]
            pvt_ps = smps[:, 88:90]
            sv_ps = smps[0:1, 90:91]
            svv_ps = smps[0:1, 91:92]
            aff_ps = smps[:, 96:97]
            trq_ps = smps[0:1, 128:192].bitcast(bf16)
            trk_ps = smps[0:1, 192:256].bitcast(bf16)
            trqk_ps = smps[0:1, 128:256].bitcast(bf16)
            sct_ps = smps[:, 384:512]

            def ts(out, in0, s1, op0, s2=None, op1=None):
                if s2 is None:
                    nc.vector.tensor_scalar(out, in0, s1, None, op0)
                else:
                    nc.vector.tensor_scalar(out, in0, s1, s2, op0, op1)

            prev_trigger_q = [None, None]
            exp_hist = []
            for b in range(L):
                bqr = spool.tile([1, TL * S], bf16, tag="bqr")
                nc.sync.dma_start(bqr[:], bqr_d[b])
                tqkv = wpool.tile([S, 3 * TL * S], bf16, tag="tqkv")
                half = 3 * TL * S // 2
                nc.sync.dma_start(tqkv[:, 0:half], tqkv_d[b][:, 0:half])
                nc.sync.dma_start(tqkv[:, half:2 * half], tqkv_d[b][:, half:2 * half])
                aux = spool.tile([S, 484], bf16, tag="aux")
                nc.scalar.dma_start(aux[:], aux_d[b])
                topo_wt = aux[:, 0:3 * S]
                small = aux[:, 3 * S:484]

                qkvb = pre[:, 138 + 32 * b:138 + 32 * b + 32]
                mt = small[:, 48:64]
                wmt = pre[:, 394 + 16 * b:394 + 16 * b + 16]
                pb = 2 + 10 * b
                topo_c = pre[:, pb:pb + 3]
                topo_bp = pre[:, pb + 3:pb + 6]
                gam = pre[:, pb + 6:pb + 7]
                bet = pre[:, pb + 7:pb + 8]
                g1h = pre[:, pb + 8:pb + 9]
                wbc = pre[:, pb + 9:pb + 10]


                # ---- remote-send descriptor gen for this batch (early, idle
                # Pool time; data dep is deferred to the trigger) ----
                if b < L - 1:
                    for d in range(1, 8):
                        rdests = [None] * 8
                        rdests[d] = (0, d)
                        pr = nc.gpsimd.remote_dma_broadcast(
                            rcv[b][:, d:d + 1], aff_sb[b][:],
                            rsems[(b, d)], lsem_q[0], rdests=rdests)
                        if prev_trigger_q[0] is not None:
                            dep = InstructionNameOrderedSet()
                            dep.add(prev_trigger_q[0].ins.name)
                            pr.ins.add_nosync_dependencies_from(dep)

                # ---- acquire v ----
                if b == 0:
                    nc.vector.tensor_copy(v_col[:], pre[:, 0:1])
                else:
                    asm = work.tile([S, 8], fp32, tag="asm")
                    prev_w = prev_selfcopy
                    for d in range(1, 8):
                        wi = nc.vector.wait_ge(rsems[(b - 1, d)], 0)
                        asm_waits.append((wi, rsems[(b - 1, d)], 2))
                        wdep = InstructionNameOrderedSet()
                        wdep.add(prev_w.ins.name)
                        wi.ins.add_nosync_dependencies_from(wdep)
                        prev_w = wi
                    mi = nc.vector.tensor_mul(
                        asm[:], rcv[b - 1][:],
                        pre[:, 82 + 8 * (b - 1):82 + 8 * b])
                    mdep = InstructionNameOrderedSet()
                    mdep.add(prev_w.ins.name)
                    mi.ins.add_nosync_dependencies_from(mdep)
                    xg = work.tile([S, 1], fp32, tag="xg")
                    nc.vector.tensor_reduce(xg[:], asm[:], AX, add_op)
                    # adaptive gelu of previous batch (gain pre-folded in selg)
                    t1 = work.tile([S, 1], fp32, tag="t1")
                    nc.vector.tensor_mul(t1[:], xg[:], xg[:])
                    nc.vector.tensor_mul(t1[:], t1[:], xg[:])
                    nc.vector.scalar_tensor_tensor(t1[:], t1[:], GA, xg[:],
                                                   mul_op, add_op)
                    nc.scalar.activation(t1[:], t1[:], Tanh, scale=GC)
                    nc.vector.scalar_tensor_tensor(t1[:], t1[:], 1.0, xg[:],
                                                   add_op, mul_op)
                    nc.vector.tensor_mul(v_col[:], t1[:], g1h)

                # ---- stats + Newton rsqrt ----
                nc.tensor.matmul(sv_ps, ones_col[:], v_col[:], start=True, stop=True)
                nc.tensor.matmul(svv_ps, v_col[:], v_col[:], start=True, stop=True)
                ts(sc[:, 0:1], sv_ps, 1.0 / S, mul_op)
                ts(sc[:, 1:2], svv_ps, 1.0 / S, mul_op)
                nc.vector.scalar_tensor_tensor(sc[:, 3:4], sc[:, 0:1], sc[:, 0:1],
                                               sc[:, 1:2], mul_op, sub_op)
                ts(sc[:, 4:5], sc[:, 3:4], -1.0, mul_op, EPS, add_op)
                ts(sc[:, 5:6], sc[:, 3:4], -0.5, mul_op, 0.5 * EPS, add_op)
                ts(yBi, sci[:, 4:5], 1, shr_op)
                nc.vector.tensor_sub(yAi, magic[:, 0:1], yBi)
                for _ in range(1):
                    nc.vector.scalar_tensor_tensor(yB[:], yA[:], sc[:, 5:6],
                                                   yA[:], mul_op, mul_op)
                    ts(yB[:], yB[:], -1.0, mul_op, 1.5, add_op)
                    nc.vector.tensor_mul(yA[:], yA[:], yB[:])
                nc.vector.tensor_copy(sc[:, 6:7], yA[:])
                nc.vector.tensor_mul(sc[:, 7:8], yA[:], sc[:, 0:1])
                nc.tensor.matmul(bc_ps, ones_row[:], sc[:, 5:8], start=True, stop=True)
                nc.vector.tensor_copy(bc_sb[:], bc_ps)
                rstd_c = bc_sb[:, 1:2]
                murstd_c = bc_sb[:, 2:3]

                # ---- u = rstd*gamma*(v-mu) + beta ----
                gv = work.tile([S, 1], fp32, tag="gv")
                gm = work.tile([S, 1], fp32, tag="gm")
                nc.vector.tensor_mul(gv[:], v_col[:], gam)
                ts(gm[:], gam, murstd_c, mul_op)
                nc.vector.scalar_tensor_tensor(u_col[:], gv[:], rstd_c, gm[:],
                                               mul_op, sub_op)
                nc.vector.tensor_add(u_col[:], u_col[:], bet)
                nc.vector.tensor_copy(v_bf[:], v_col[:])

                # ---- topo qkv on raw v (gamma folded into weights) ----
                for m in range(3):
                    nc.tensor.matmul(A_ps[:, m:m + 1], topo_wt[:, m * S:(m + 1) * S],
                                     v_bf[:], start=True, stop=True)
                cm = work.tile([S, 3], fp32, tag="cm")
                ts(cm[:], topo_c, murstd_c, mul_op)
                nc.vector.scalar_tensor_tensor(cm[:], A_ps, rstd_c, cm[:],
                                               mul_op, sub_op)
                nc.vector.tensor_add(qkvt_bf[:], cm[:], topo_bp)

                # ---- topo attention (outer product via column transposes) ----
                nc.tensor.transpose(trq_ps, qkvt_bf[:, 0:1], identb[:])
                nc.tensor.transpose(trk_ps, qkvt_bf[:, 1:2], identb[:])
                nc.vector.tensor_copy(qkrow[:], trqk_ps)
                nc.tensor.matmul(sct_ps, qkrow[0:1, S:2 * S], qkrow[0:1, 0:S],
                                 start=True, stop=True)
                nc.scalar.activation(Et_sb[:], sct_ps, Exp)
                nc.vector.tensor_copy(pvr_t[:, 0:1], qkvt_bf[:, 2:3])
                nc.tensor.matmul(pvt_ps, Et_sb[:], pvr_t[:], start=True, stop=True)
                rd1 = work.tile([S, 1], fp32, tag="rd1")
                nc.vector.reciprocal(rd1[:], pvt_ps[:, 1:2])
                nc.vector.tensor_mul(up_col[:], pvt_ps[:, 0:1], rd1[:])
                nc.vector.tensor_add(up_col[:], up_col[:], u_col[:])
                nc.vector.tensor_copy(up_bf[:], up_col[:])

                # ---- neuron k/v matvecs (q rides the broadcast matmuls) ----
                for mtt in range(16, 48):
                    nc.tensor.matmul(qkv_ps[:, mtt - 16:mtt - 15],
                                     tqkv[:, mtt * S:(mtt + 1) * S],
                                     up_bf[:], start=True, stop=True)
                nc.vector.tensor_add(qkv_sb[:], qkv_ps[:, 0:32], qkvb)

                # masked v and mask columns for PV
                p2 = pvr_n[:].rearrange("p (t k) -> p t k", k=2)
                nc.vector.tensor_mul(p2[:, :, 0], qkv_sb[:, 16:32], mt)
                nc.vector.tensor_copy(p2[:, :, 1], mt)

                # ---- Q broadcast (+ q bias row), k-scale, exp, PV ----
                nc.vector.tensor_copy(Ub[:], up_col[:].broadcast_to([S, S]))
                for bank in range(4):
                    for j in range(4):
                        t = 4 * bank + j
                        nc.tensor.matmul(
                            sbank[bank][:, j * S:(j + 1) * S],
                            Ub[:], tqkv[:, t * S:(t + 1) * S],
                            start=(j == 0), stop=False,
                            skip_group_check=True)
                    nc.tensor.matmul(
                        sbank[bank][:], ones_row_bf[:],
                        bqr[:, bank * 512:(bank + 1) * 512],
                        start=False, stop=True, skip_group_check=True)
                    t0 = 4 * bank
                    kb = qkv_sb[:, t0:t0 + 4].unsqueeze(2) \
                        .broadcast_to([S, 4, S])
                    nc.vector.tensor_mul(
                        sc_ch[bank][:].rearrange("p (t j) -> p t j", j=S),
                        sbank[bank][:].rearrange("p (t j) -> p t j", j=S), kb)
                    ei = nc.scalar.activation(E_ch[bank][:], sc_ch[bank][:],
                                              Exp)
                    if bank == 3:
                        exp_hist.append(ei)
                    for j in range(4):
                        t = t0 + j
                        nc.tensor.matmul(pvn_ps[:, 2 * t:2 * t + 2],
                                         E_ch[bank][:, j * S:(j + 1) * S],
                                         pvr_n[:, 2 * t:2 * t + 2],
                                         start=True, stop=True)

                pv2 = pvn_ps.rearrange("p (t k) -> p t k", k=2)
                nc.vector.reciprocal(rden[:], pv2[:, :, 1])
                nc.vector.tensor_mul(zp[:], pv2[:, :, 0], rden[:])

                # ---- aff_rep[p] = sum_s wmt[s,p%16]*(zp+u')[s,p%16] + wbc ----
                wmt_r = wmt.unsqueeze(1).broadcast_to([S, 8, TL])
                zp_r = zp[:].unsqueeze(1).broadcast_to([S, 8, TL])
                nc.vector.scalar_tensor_tensor(
                    af_big[:].rearrange("p (r t) -> p r t", r=8),
                    zp_r, up_col[:], wmt_r, add_op, mul_op)
                nc.tensor.matmul(aff_ps, af_big[:], ones_col[:],
                                 start=True, stop=True)
                aff_w = nc.vector.scalar_tensor_tensor(aff_sb[b][:], aff_ps,
                                                       1.0, wbc, bypass, add_op)

                if b < L - 1:
                    prev_selfcopy = nc.vector.tensor_copy(rcv[b][:, 0:1],
                                                          aff_sb[b][:])
                    # count register derived from aff bits: gives the trigger a
                    # trace-time data dependency on the aff write while the
                    # descriptor-gen preps stay early.
                    aff_i = aff_sb[b][:].bitcast(int32)
                    nc.vector.tensor_scalar(cnt_cells[b][:], aff_i[0:1, 0:1],
                                            0, 7, and_op, or_op)
                    nc.gpsimd.load(cnt_regs[b], cnt_cells[b][:])
                    prev_trigger_q[0] = nc.gpsimd.trigger_dma(
                        count=cnt_regs[b])
                else:
                    nc.sync.dma_start(out_d, aff_sb[b][0:TL, 0:1])

    for inst, sem, val in asm_waits:
        patched = False
        for w in inst.ins.sync_info.on_wait:
            if w.id == sem.num:
                w.wait_value = val
                patched = True
        assert patched, f"lost remote-sem wait on {inst.ins.name}"

    nc.compile()
    return nc


def _host_prep(x, W, mask, attn_t, attn_n, norm_params, ada):
    import ml_dtypes
    f32 = np.float32
    bf16 = ml_dtypes.bfloat16
    x, W, mask, attn_t, attn_n, norm_params, ada = (
        np.ascontiguousarray(np.asarray(a, f32))
        for a in (x, W, mask, attn_t, attn_n, norm_params, ada))
    P = _nc_perm()
    Pinv = [P.index(i) for i in range(8)]

    gamma = norm_params[:, 0, :]
    beta = norm_params[:, 1, :]
    topo_w = attn_t[:, :, :, :S]
    topo_b = attn_t[:, :, :, S]
    topo_wg = topo_w * gamma[:, None, None, :]          # (L,3,i,s)
    topo_c = topo_wg.sum(axis=3)                        # (L,3,i)
    topo_bp = np.einsum('lmis,ls->lmi', topo_w, beta) + topo_b
    # fold attention scale into q
    topo_wg[:, 0] *= RS
    topo_c[:, 0] *= RS
    topo_bp[:, 0] *= RS

    wmat = W[:, :, :S] * mask
    wbias = W[:, :, S]

    sel = np.zeros((S, 8), f32)
    for p in range(S):
        sel[p, p // 16] = 1.0
    magic = np.array([[MAGIC, 0]], np.int32)

    in_maps = []
    for c in range(N_CORES):
        perm = np.array([16 * Pinv[P[c] ^ (p >> 4)] + (p & 15)
                         for p in range(S)], np.int64)
        sl = slice(c * TL, (c + 1) * TL)

        an = attn_n[:, sl]                               # (L,TL,3,o,i)
        anw = an[:, :, :, :, :S][:, :, :, perm][:, :, :, :, perm]
        anw[:, :, 0] *= RS
        # -> [b, s_in, m, t, s_out]
        tqkv = np.ascontiguousarray(
            anw.transpose(0, 4, 2, 1, 3)).reshape(L, S, 3 * TL * S).astype(bf16)

        anb = an[:, :, :, perm, S]                       # (L,TL,3,o)
        anb[:, :, 0] *= RS
        kvb = np.ascontiguousarray(
            anb[:, :, 1:3].transpose(0, 3, 2, 1)).reshape(L, S, 32)
        bqr = np.ascontiguousarray(anb[:, :, 0, :]).reshape(L, 1, TL * S).astype(bf16)

        topo_wt = np.ascontiguousarray(
            topo_wg[:, :, perm][:, :, :, perm].transpose(0, 3, 1, 2)
        ).reshape(L, S, 3 * S)

        small = np.zeros((L, S, 100), f32)
        small[:, :, 0:32] = kvb
        small[:, :, 48:64] = mask[:, sl][:, :, perm].transpose(0, 2, 1)
        small[:, :, 64:80] = wmat[:, sl][:, :, perm].transpose(0, 2, 1)
        small[:, :, 80:83] = topo_c[:, :, perm].transpose(0, 2, 1)
        small[:, :, 83:86] = topo_bp[:, :, perm].transpose(0, 2, 1)
        small[:, :, 86] = gamma[:, perm]
        small[:, :, 87] = beta[:, perm]
        # receive-side adaptive gelu params of the PREVIOUS batch
        small[1:, :, 88] = ada[:L - 1, :, 0][:, perm]
        small[1:, :, 89] = 0.5 * ada[:L - 1, :, 1][:, perm]
        small[1:, :, 92:100] = sel[None] * ada[:L - 1, :, 0][:, perm][:, :, None]
        # replicated affine bias of own neurons
        tloc = np.arange(S) % TL + c * TL
        small[:, :, 90] = wbias[:, tloc]

        pre = np.zeros((S, 522), f32)
        pre[:, 0] = x[perm]
        for b in range(L):
            pb = 2 + 10 * b
            pre[:, pb:pb + 3] = topo_c[b][:, perm].T
            pre[:, pb + 3:pb + 6] = topo_bp[b][:, perm].T
            pre[:, pb + 6] = gamma[b, perm]
            pre[:, pb + 7] = beta[b, perm]
            if b >= 1:
                pre[:, pb + 8] = 0.5 * ada[b - 1, perm, 1]
                pre[:, 82 + 8 * (b - 1):82 + 8 * b] = \
                    sel * ada[b - 1, perm, 0][:, None]
            pre[:, pb + 9] = wbias[b, tloc]
            pre[:, 138 + 32 * b:138 + 32 * b + 32] = kvb[b]
            pre[:, 394 + 16 * b:394 + 16 * b + 16] = wmat[b, sl][:, perm].T

        identb = np.eye(S, dtype=bf16)

        aux = np.concatenate([topo_wt, small], axis=2).astype(bf16)
        in_maps.append(dict(tqkv=tqkv, aux=aux, pre=pre,
                            bqr=bqr, identb=identb, magic=magic))
    return in_maps


def kernel(x, W, mask, attn_t, attn_n, attn_mask_n, norm_params, ada,
           span_ids, tb_ids):
    global _cached
    from concourse import bass_utils
    _nc_perm()
    if _cached is None:
        _cached = _build()
    nc = _cached
    in_maps = _host_prep(x, W, mask, attn_t, attn_n, norm_params, ada)
    res = bass_utils.run_bass_kernel_spmd(nc, in_maps, core_ids=list(range(N_CORES)))
    out = np.concatenate([res.results[c]["out"].reshape(TL) for c in range(N_CORES)])
    return out.astype(np.float32)
